# revision 9
# baseline (speedup 1.0000x reference)
"""Additive attention kernel for Trainium2, data-parallel over batch on 8 cores.

Computation (per batch b):
  x3 = W_conv1 @ x_b            # [HID, HW]  (1x1 conv, contract over C)
  h3 = W_lin @ h_b + b_lin      # [HID]
  a1 = tanh(x3 + h3[:, None])   # [HID, HW]
  a2 = W_attn @ a1              # [HW]
  a3 = softmax(a2)              # [HW]
  ctx = x_b @ a3                # [C]
Outputs: (a3 [B, HW], ctx [B, C]).

Per-core mapping (4 batches/core):
  - conv as PE matmuls: lhsT = W_conv1.T chunks [128(C), 128(HID)], rhs = x
    chunks [128(C), 512(HW)], accumulate over 4 C-chunks in PSUM.
  - tanh+bias on ACT (bias = h3 per-partition column).
  - a2 via PE: lhsT = W_attn chunk [128, 1], rhs = a1 [128, 512], M=1 rows
    landing in psum partition b (one [4, HW] psum tile holds all batches).
  - softmax per batch on row [1, HW]: DVE max, ACT exp (+accum_out -> Z),
    DVE reciprocal, ACT scale-copy.
  - a3 broadcast across partitions via PE: ones[1,128].T @ a3[1,512].
  - ctx via fused DVE tensor_tensor_reduce: (x * bcast) summed over free dim.
"""

import os
import sys

sys.path.insert(0, "/opt/trn_rl_repo")

from contextlib import ExitStack

import numpy as np

import concourse.bacc as bacc
import concourse.mybir as mybir
import concourse.tile as tile
from concourse.bass_utils import run_bass_kernel_spmd

B, C, HGT, WID = 32, 512, 32, 32
HW = HGT * WID
HID, MEM = 256, 256
NCORES = 8
BL = B // NCORES  # batches per core
P = 128
KC = C // P       # C chunks (contract dim of conv)
MC = HID // P     # HID chunks
CC = C // P       # C chunks of ctx output
NSPLIT = 512      # fp32 moving-operand max free dim
NH = HW // NSPLIT

F32 = mybir.dt.float32
# PE dtype for the big matmuls: float32r streams 1 row/cycle vs 4 for
# float32 (same 4-byte layout, reduced internal precision).
MM_DT = getattr(mybir.dt, os.environ.get("ADDATT_MM_DT", "float32r"))
BC_DT = getattr(mybir.dt, os.environ.get("ADDATT_BC_DT", "float32r"))

_CACHE = {}


def build_nc():
    nc = bacc.Bacc("TRN2", target_bir_lowering=False, debug=False,
                   num_devices=NCORES)
    x_d = nc.dram_tensor("x", [BL, C, HW], F32, kind="ExternalInput").ap()
    w1t_d = nc.dram_tensor("w1t", [C, HID], F32, kind="ExternalInput").ap()
    wlt_d = nc.dram_tensor("wlt", [MEM, HID], F32, kind="ExternalInput").ap()
    ht_d = nc.dram_tensor("ht", [MEM, BL], F32, kind="ExternalInput").ap()
    bl_d = nc.dram_tensor("bl", [HID, 1], F32, kind="ExternalInput").ap()
    wa_d = nc.dram_tensor("wa", [HID, 1], F32, kind="ExternalInput").ap()
    a3_d = nc.dram_tensor("a3", [BL, HW], F32, kind="ExternalOutput").ap()
    ctx_d = nc.dram_tensor("ctx", [BL, C], F32, kind="ExternalOutput").ap()

    Act = mybir.ActivationFunctionType
    Alu = mybir.AluOpType

    with tile.TileContext(nc) as tc, ExitStack() as ctx:
        consts = ctx.enter_context(tc.tile_pool(name="consts", bufs=1))
        xpool = ctx.enter_context(tc.tile_pool(name="xp", bufs=BL))
        a1pool = ctx.enter_context(tc.tile_pool(name="a1p", bufs=4))
        scrpool = ctx.enter_context(tc.tile_pool(name="scr", bufs=2))
        smalls = ctx.enter_context(tc.tile_pool(name="smalls", bufs=1))
        convps = ctx.enter_context(tc.tile_pool(name="convps", bufs=2, space="PSUM"))
        a2psp = ctx.enter_context(tc.tile_pool(name="a2psp", bufs=1, space="PSUM"))
        bcpsp = ctx.enter_context(tc.tile_pool(name="bcpsp", bufs=1, space="PSUM"))

        # ---- constants / weights ----
        w1t_s = consts.tile([P, KC, HID], F32)
        nc.sync.dma_start(out=w1t_s[:], in_=w1t_d.rearrange("(kc p) m -> p kc m", p=P))
        wlt_s = consts.tile([P, MEM // P, HID], F32)
        nc.sync.dma_start(out=wlt_s[:], in_=wlt_d.rearrange("(kc p) m -> p kc m", p=P))
        ht_s = consts.tile([P, MEM // P, BL], F32)
        nc.sync.dma_start(out=ht_s[:], in_=ht_d.rearrange("(kc p) b -> p kc b", p=P))
        bl_s = consts.tile([P, MC, 1], F32)
        nc.sync.dma_start(out=bl_s[:], in_=bl_d.rearrange("(mc p) o -> p mc o", p=P))
        wa_s = consts.tile([P, MC, 1], F32)
        nc.sync.dma_start(out=wa_s[:], in_=wa_d.rearrange("(mc p) o -> p mc o", p=P))
        ones_s = consts.tile([P, P], F32)
        nc.vector.memset(ones_s[:], 1.0)

        # ---- x loads (one big DMA per batch) ----
        xb = []
        for b in range(BL):
            xt = xpool.tile([P, KC, HW], F32, tag="xb")
            nc.sync.dma_start(out=xt[:], in_=x_d[b].rearrange("(kc p) s -> p kc s", p=P))
            xb.append(xt)

        # ---- h3 = W_lin @ h + b_lin (tiny) ----
        h3_s = smalls.tile([P, MC, BL], F32)
        for mc in range(MC):
            h3ps = convps.tile([P, BL], F32, tag="cps")
            for kc in range(MEM // P):
                nc.tensor.matmul(
                    h3ps[:],
                    lhsT=wlt_s[:, kc, mc * P:(mc + 1) * P],
                    rhs=ht_s[:, kc, :],
                    start=(kc == 0), stop=(kc == MEM // P - 1),
                )
            nc.vector.tensor_scalar_add(h3_s[:, mc, :], h3ps[:], bl_s[:, mc, :])

        # persistent tiles for attention rows; batch b lives on partition
        # 32*b (matmul PSUM outputs must start at partition 0/32/64/96)
        a2ps = a2psp.tile([P, HW], F32)
        p_sb = smalls.tile([P, HW], F32)
        a3_sb = smalls.tile([P, HW], F32)
        nmax_sb = smalls.tile([P, 1], F32)
        z_sb = smalls.tile([P, 1], F32)
        rz_sb = smalls.tile([P, 1], F32)

        for b in range(BL):
            # conv + tanh
            a1t = []
            for mc in range(MC):
                cps = convps.tile([P, HW], F32, tag="cps")
                for nh in range(NH):
                    ns = slice(nh * NSPLIT, (nh + 1) * NSPLIT)
                    for kc in range(KC):
                        nc.tensor.matmul(
                            cps[:, ns],
                            lhsT=w1t_s[:, kc, mc * P:(mc + 1) * P].bitcast(MM_DT),
                            rhs=xb[b][:, kc, ns].bitcast(MM_DT),
                            start=(kc == 0), stop=(kc == KC - 1),
                        )
                a1 = a1pool.tile([P, HW], F32, tag="a1")
                nc.scalar.activation(a1[:], cps[:], Act.Tanh,
                                     bias=h3_s[:, mc, b:b + 1])
                a1t.append(a1)

            # a2 row for this batch. Matmul outputs/lhsT must sit at base
            # partition 0/32/64, so batch 3 reuses row 0 (WAR handled by Tile).
            rr = 32 * (b % 3)
            rb = slice(rr, rr + 1)
            for nh in range(NH):
                ns = slice(nh * NSPLIT, (nh + 1) * NSPLIT)
                for mc in range(MC):
                    nc.tensor.matmul(
                        a2ps[rb, ns],
                        lhsT=wa_s[:, mc, :].bitcast(MM_DT),
                        rhs=a1t[mc][:, ns].bitcast(MM_DT),
                        start=(mc == 0), stop=(mc == MC - 1),
                    )

            # softmax on row b
            nc.vector.reduce_max(nmax_sb[rb, :], a2ps[rb, :],
                                 axis=mybir.AxisListType.X, negate=True)
            nc.scalar.activation(p_sb[rb, :], a2ps[rb, :], Act.Exp,
                                 bias=nmax_sb[rb, :],
                                 accum_out=z_sb[rb, :])
            nc.vector.reciprocal(rz_sb[rb, :], z_sb[rb, :])
            nc.scalar.mul(a3_sb[rb, :], p_sb[rb, :], rz_sb[rb, :])
            nc.sync.dma_start(out=a3_d[b:b + 1, :], in_=a3_sb[rb, :])

            # broadcast a3 row across partitions via PE, then fused mul+reduce
            bcps = bcpsp.tile([P, HW], F32, tag="bc")
            for nh in range(NH):
                ns = slice(nh * NSPLIT, (nh + 1) * NSPLIT)
                nc.tensor.matmul(
                    bcps[:, ns],
                    lhsT=ones_s[rb, :].bitcast(BC_DT),
                    rhs=a3_sb[rb, ns].bitcast(BC_DT),
                    start=True, stop=True,
                )
            # (tensor_tensor_reduce crashes the exec unit on this stack, so
            # mul and reduce are separate; reduces alternate ACT/DVE)
            ctx_sb = smalls.tile([P, CC], F32, tag=f"ctx{b}")
            for cc in range(CC):
                scr = scrpool.tile([P, HW], F32, tag="scr")
                nc.vector.tensor_mul(scr[:], xb[b][:, cc, :], bcps[:])
                if cc % 2 == 0:
                    nc.scalar.activation(scr[:], scr[:], Act.Copy,
                                         accum_out=ctx_sb[:, cc:cc + 1])
                else:
                    nc.vector.reduce_sum(ctx_sb[:, cc:cc + 1], scr[:],
                                         axis=mybir.AxisListType.X)
            nc.sync.dma_start(out=ctx_d[b].rearrange("(cc p) -> p cc", p=P),
                              in_=ctx_sb[:])

    nc.compile()
    return nc


def _get_nc():
    if "nc" not in _CACHE:
        _CACHE["nc"] = build_nc()
    return _CACHE["nc"]


def kernel(x, h, W_conv1, W_lin, b_lin, W_attn):
    x = np.asarray(x, dtype=np.float32)
    h = np.asarray(h, dtype=np.float32)
    W_conv1 = np.asarray(W_conv1, dtype=np.float32)
    W_lin = np.asarray(W_lin, dtype=np.float32)
    b_lin = np.asarray(b_lin, dtype=np.float32)
    W_attn = np.asarray(W_attn, dtype=np.float32)

    nc = _get_nc()

    x_r = x.reshape(B, C, HW)
    w1t = np.ascontiguousarray(W_conv1.T)
    wlt = np.ascontiguousarray(W_lin.T)
    ht = np.ascontiguousarray(h.T)
    bl = np.ascontiguousarray(b_lin.reshape(HID, 1))
    wa = np.ascontiguousarray(W_attn.reshape(HID, 1))

    in_maps = []
    for i in range(NCORES):
        sl = slice(i * BL, (i + 1) * BL)
        in_maps.append({
            "x": np.ascontiguousarray(x_r[sl]),
            "w1t": w1t,
            "wlt": wlt,
            "ht": np.ascontiguousarray(ht[:, sl]),
            "bl": bl,
            "wa": wa,
        })

    res = run_bass_kernel_spmd(nc, in_maps, core_ids=list(range(NCORES)))
    a3 = np.concatenate([r["a3"] for r in res.results], axis=0)
    ctx = np.concatenate([r["ctx"] for r in res.results], axis=0)
    return a3, ctx


# revision 12
# speedup vs baseline: 1.7414x; 1.7414x over previous
"""Additive attention kernel for Trainium2, data-parallel over batch on 8 cores.

Computation (per batch b):
  x3 = W_conv1 @ x_b            # [HID, HW]  (1x1 conv, contract over C)
  h3 = W_lin @ h_b + b_lin      # [HID]
  a1 = tanh(x3 + h3[:, None])   # [HID, HW]
  a2 = W_attn @ a1              # [HW]
  a3 = softmax(a2)              # [HW]
  ctx = x_b @ a3                # [C]
Outputs: (a3 [B, HW], ctx [B, C]).

Per-core mapping (4 batches/core):
  - conv as PE matmuls: lhsT = W_conv1.T chunks [128(C), 128(HID)], rhs = x
    chunks [128(C), 512(HW)], accumulate over 4 C-chunks in PSUM.
  - tanh+bias on ACT (bias = h3 per-partition column).
  - a2 via PE: lhsT = W_attn chunk [128, 1], rhs = a1 [128, 512], M=1 rows
    landing in psum partition b (one [4, HW] psum tile holds all batches).
  - softmax per batch on row [1, HW]: DVE max, ACT exp (+accum_out -> Z),
    DVE reciprocal, ACT scale-copy.
  - a3 broadcast across partitions via PE: ones[1,128].T @ a3[1,512].
  - ctx via fused DVE tensor_tensor_reduce: (x * bcast) summed over free dim.
"""

import os
import sys

sys.path.insert(0, "/opt/trn_rl_repo")

from contextlib import ExitStack

import numpy as np

import concourse.bacc as bacc
import concourse.mybir as mybir
import concourse.tile as tile
from concourse.bass_utils import run_bass_kernel_spmd

B, C, HGT, WID = 32, 512, 32, 32
HW = HGT * WID
HID, MEM = 256, 256
NCORES = 8
BL = B // NCORES  # batches per core
P = 128
KC = C // P       # C chunks (contract dim of conv)
MC = HID // P     # HID chunks
CC = C // P       # C chunks of ctx output
NSPLIT = 512      # fp32 moving-operand max free dim
NH = HW // NSPLIT

F32 = mybir.dt.float32
# PE dtype for the big matmuls: float32r streams 1 row/cycle vs 4 for
# float32 (same 4-byte layout, reduced internal precision).
CONV_DT = getattr(mybir.dt, os.environ.get("ADDATT_CONV_DT", "float32r"))
A2_DT = getattr(mybir.dt, os.environ.get("ADDATT_A2_DT", "float32"))
BC_DT = getattr(mybir.dt, os.environ.get("ADDATT_BC_DT", "float32"))

_CACHE = {}


def build_nc():
    nc = bacc.Bacc("TRN2", target_bir_lowering=False, debug=False,
                   num_devices=NCORES)
    XDT = CONV_DT if CONV_DT == mybir.dt.float32r else F32
    x_d = nc.dram_tensor("x", [BL, C, HW], XDT, kind="ExternalInput").ap()
    w1t_d = nc.dram_tensor("w1t", [C, HID], XDT, kind="ExternalInput").ap()
    wlt_d = nc.dram_tensor("wlt", [MEM, HID], F32, kind="ExternalInput").ap()
    ht_d = nc.dram_tensor("ht", [MEM, BL], F32, kind="ExternalInput").ap()
    bl_d = nc.dram_tensor("bl", [HID, 1], F32, kind="ExternalInput").ap()
    wa_d = nc.dram_tensor("wa", [HID, 1], F32, kind="ExternalInput").ap()
    a3_d = nc.dram_tensor("a3", [BL, HW], F32, kind="ExternalOutput").ap()
    ctx_d = nc.dram_tensor("ctx", [BL, C], F32, kind="ExternalOutput").ap()

    Act = mybir.ActivationFunctionType
    Alu = mybir.AluOpType

    with tile.TileContext(nc) as tc, ExitStack() as ctx:
        consts = ctx.enter_context(tc.tile_pool(name="consts", bufs=1))
        xpool = ctx.enter_context(tc.tile_pool(name="xp", bufs=BL))
        a1pool = ctx.enter_context(tc.tile_pool(name="a1p", bufs=4))
        scrpool = ctx.enter_context(tc.tile_pool(name="scr", bufs=2))
        smalls = ctx.enter_context(tc.tile_pool(name="smalls", bufs=1))
        convps = ctx.enter_context(tc.tile_pool(name="convps", bufs=2, space="PSUM"))
        a2psp = ctx.enter_context(tc.tile_pool(name="a2psp", bufs=1, space="PSUM"))
        bcpsp = ctx.enter_context(tc.tile_pool(name="bcpsp", bufs=1, space="PSUM"))

        # ---- constants / weights ----
        w1t_s = consts.tile([P, KC, HID], XDT)
        nc.sync.dma_start(out=w1t_s[:], in_=w1t_d.rearrange("(kc p) m -> p kc m", p=P))
        wlt_s = consts.tile([P, MEM // P, HID], F32)
        nc.sync.dma_start(out=wlt_s[:], in_=wlt_d.rearrange("(kc p) m -> p kc m", p=P))
        ht_s = consts.tile([P, MEM // P, BL], F32)
        nc.sync.dma_start(out=ht_s[:], in_=ht_d.rearrange("(kc p) b -> p kc b", p=P))
        bl_s = consts.tile([P, MC, 1], F32)
        nc.sync.dma_start(out=bl_s[:], in_=bl_d.rearrange("(mc p) o -> p mc o", p=P))
        wa_s = consts.tile([P, MC, 1], F32)
        nc.sync.dma_start(out=wa_s[:], in_=wa_d.rearrange("(mc p) o -> p mc o", p=P))
        ones_s = consts.tile([P, P], F32)
        nc.vector.memset(ones_s[:], 1.0)

        # ---- x loads (one big DMA per batch) ----
        xb = []
        for b in range(BL):
            xt = xpool.tile([P, KC, HW], XDT, tag="xb")
            nc.sync.dma_start(out=xt[:], in_=x_d[b].rearrange("(kc p) s -> p kc s", p=P))
            xb.append(xt)

        # ---- h3 = W_lin @ h + b_lin (tiny) ----
        h3_s = smalls.tile([P, MC, BL], F32)
        for mc in range(MC):
            h3ps = convps.tile([P, BL], F32, tag="cps")
            for kc in range(MEM // P):
                nc.tensor.matmul(
                    h3ps[:],
                    lhsT=wlt_s[:, kc, mc * P:(mc + 1) * P],
                    rhs=ht_s[:, kc, :],
                    start=(kc == 0), stop=(kc == MEM // P - 1),
                )
            nc.vector.tensor_scalar_add(h3_s[:, mc, :], h3ps[:], bl_s[:, mc, :])

        # persistent tiles for attention rows; batch b lives on partition
        # 32*b (matmul PSUM outputs must start at partition 0/32/64/96)
        a2ps = a2psp.tile([P, HW], F32)
        p_sb = smalls.tile([P, HW], F32)
        a3_sb = smalls.tile([P, HW], F32)
        nmax_sb = smalls.tile([P, 1], F32)
        z_sb = smalls.tile([P, 1], F32)
        rz_sb = smalls.tile([P, 1], F32)

        for b in range(BL):
            # conv + tanh
            a1t = []
            for mc in range(MC):
                cps = convps.tile([P, HW], F32, tag="cps")
                for nh in range(NH):
                    ns = slice(nh * NSPLIT, (nh + 1) * NSPLIT)
                    for kc in range(KC):
                        nc.tensor.matmul(
                            cps[:, ns],
                            lhsT=w1t_s[:, kc, mc * P:(mc + 1) * P],
                            rhs=xb[b][:, kc, ns],
                            start=(kc == 0), stop=(kc == KC - 1),
                        )
                a1 = a1pool.tile([P, HW], F32, tag="a1")
                nc.scalar.activation(a1[:], cps[:], Act.Tanh,
                                     bias=h3_s[:, mc, b:b + 1])
                a1t.append(a1)

            # a2 row for this batch. Matmul outputs/lhsT must sit at base
            # partition 0/32/64, so batch 3 reuses row 0 (WAR handled by Tile).
            rr = 32 * (b % 3)
            rb = slice(rr, rr + 1)
            for nh in range(NH):
                ns = slice(nh * NSPLIT, (nh + 1) * NSPLIT)
                for mc in range(MC):
                    nc.tensor.matmul(
                        a2ps[rb, ns],
                        lhsT=wa_s[:, mc, :].bitcast(A2_DT),
                        rhs=a1t[mc][:, ns].bitcast(A2_DT),
                        start=(mc == 0), stop=(mc == MC - 1),
                    )

            # softmax on row b
            nc.vector.reduce_max(nmax_sb[rb, :], a2ps[rb, :],
                                 axis=mybir.AxisListType.X, negate=True)
            nc.scalar.activation(p_sb[rb, :], a2ps[rb, :], Act.Exp,
                                 bias=nmax_sb[rb, :],
                                 accum_out=z_sb[rb, :])
            nc.vector.reciprocal(rz_sb[rb, :], z_sb[rb, :])
            nc.scalar.mul(a3_sb[rb, :], p_sb[rb, :], rz_sb[rb, :])
            nc.sync.dma_start(out=a3_d[b:b + 1, :], in_=a3_sb[rb, :])

            # broadcast a3 row across partitions via PE, then fused mul+reduce
            bcps = bcpsp.tile([P, HW], F32, tag="bc")
            for nh in range(NH):
                ns = slice(nh * NSPLIT, (nh + 1) * NSPLIT)
                nc.tensor.matmul(
                    bcps[:, ns],
                    lhsT=ones_s[rb, :].bitcast(BC_DT),
                    rhs=a3_sb[rb, ns].bitcast(BC_DT),
                    start=True, stop=True,
                )
            # (tensor_tensor_reduce crashes the exec unit on this stack, so
            # mul and reduce are separate; reduces alternate ACT/DVE)
            ctx_sb = smalls.tile([P, CC], F32, tag=f"ctx{b}")
            for cc in range(CC):
                scr = scrpool.tile([P, HW], F32, tag="scr")
                nc.vector.tensor_mul(scr[:], xb[b][:, cc, :].bitcast(F32), bcps[:])
                if cc % 2 == 0:
                    nc.scalar.activation(scr[:], scr[:], Act.Copy,
                                         accum_out=ctx_sb[:, cc:cc + 1])
                else:
                    nc.vector.reduce_sum(ctx_sb[:, cc:cc + 1], scr[:],
                                         axis=mybir.AxisListType.X)
            nc.sync.dma_start(out=ctx_d[b].rearrange("(cc p) -> p cc", p=P),
                              in_=ctx_sb[:])

    nc.compile()
    return nc


def _get_nc():
    if "nc" not in _CACHE:
        _CACHE["nc"] = build_nc()
    return _CACHE["nc"]


def kernel(x, h, W_conv1, W_lin, b_lin, W_attn):
    x = np.asarray(x, dtype=np.float32)
    h = np.asarray(h, dtype=np.float32)
    W_conv1 = np.asarray(W_conv1, dtype=np.float32)
    W_lin = np.asarray(W_lin, dtype=np.float32)
    b_lin = np.asarray(b_lin, dtype=np.float32)
    W_attn = np.asarray(W_attn, dtype=np.float32)

    nc = _get_nc()

    x_r = x.reshape(B, C, HW)
    w1t = np.ascontiguousarray(W_conv1.T)
    wlt = np.ascontiguousarray(W_lin.T)
    ht = np.ascontiguousarray(h.T)
    bl = np.ascontiguousarray(b_lin.reshape(HID, 1))
    wa = np.ascontiguousarray(W_attn.reshape(HID, 1))

    in_maps = []
    for i in range(NCORES):
        sl = slice(i * BL, (i + 1) * BL)
        in_maps.append({
            "x": np.ascontiguousarray(x_r[sl]),
            "w1t": w1t,
            "wlt": wlt,
            "ht": np.ascontiguousarray(ht[:, sl]),
            "bl": bl,
            "wa": wa,
        })

    res = run_bass_kernel_spmd(nc, in_maps, core_ids=list(range(NCORES)))
    a3 = np.concatenate([r["a3"] for r in res.results], axis=0)
    ctx = np.concatenate([r["ctx"] for r in res.results], axis=0)
    return a3, ctx
